# revision 7
# baseline (speedup 1.0000x reference)
"""Trainium2 Bass kernel for the Debiased Representation Loss.

Strategy (8 NeuronCores, SPMD):
  - Shard the batch B=4096 into 8 x 512 rows. Each core computes its row-block
    of the three B x B matrices (base similarity -> mask, z similarity, f1/f2
    attention logits) tile-by-tile in SBUF/PSUM (never materialized in HBM),
    the masked attention softmax-weighted contrastive row losses, and partial
    column sums of the classifier softmax probs.
  - Inputs are replicated (host pre-normalizes + casts to bf16 and transposes
    to feature-major layout). Each core's input columns are rotated so its own
    rows sit at local columns 0..511 -> the diagonal block position is the
    same on every core (reductions over columns are permutation invariant),
    so a single SPMD program works with no dynamic addressing.
  - Per-core outputs are tiny partial scalars; host sums them and applies the
    closed-form entropy term (O(200) flops).
"""

import sys

sys.path.insert(0, "/opt/trn_rl_repo")

import math
import numpy as np
import ml_dtypes

N_CORES = 8
B, D, HID = 4096, 768, 256
R = B // N_CORES            # 512 rows per core
RBS = R // 128              # 4 row blocks per core
KT = D // 128               # 6 feature k-tiles
CT = B // 512               # 8 column tiles of 512
HT = HID // 128             # 2 hid k-tiles
NCLS = 200                  # active classes (old 100 + new 100)
TAU = 0.1
EPSILON = 0.05
EPS = 1e-8
C_SENT = 1.0e4              # sentinel offset for masked attention exp

BF = ml_dtypes.bfloat16

_CACHE = {}


def _build_module():
    from concourse import bacc, mybir, tile

    f32 = mybir.dt.float32
    bf16 = mybir.dt.bfloat16
    Alu = mybir.AluOpType
    Act = mybir.ActivationFunctionType
    Ax = mybir.AxisListType

    nc = bacc.Bacc(
        "TRN2", target_bir_lowering=False, debug=False, num_devices=N_CORES
    )

    # ---- DRAM I/O ----
    d_znc = nc.dram_tensor("zn_cols", [D, B], bf16, kind="ExternalInput")
    d_bnc = nc.dram_tensor("bn_cols", [D, B], bf16, kind="ExternalInput")
    d_zc = nc.dram_tensor("z_cols", [D, B], bf16, kind="ExternalInput")
    d_znr = nc.dram_tensor("zn_rows", [D, R], bf16, kind="ExternalInput")
    d_bnr = nc.dram_tensor("bn_rows", [D, R], bf16, kind="ExternalInput")
    d_zr = nc.dram_tensor("z_rows", [D, R], bf16, kind="ExternalInput")
    d_f1wT = nc.dram_tensor("f1wT", [D, HID], bf16, kind="ExternalInput")
    d_f2wT = nc.dram_tensor("f2wT", [D, HID], bf16, kind="ExternalInput")
    d_f1b = nc.dram_tensor("f1b", [HID, 1], f32, kind="ExternalInput")
    d_f2b = nc.dram_tensor("f2b", [HID, 1], f32, kind="ExternalInput")
    d_log = nc.dram_tensor("logits_r", [R, NCLS], f32, kind="ExternalInput")
    d_eye = nc.dram_tensor("eye128", [128, 128], bf16, kind="ExternalInput")
    d_neye = nc.dram_tensor("oneminuseye", [128, 128], bf16, kind="ExternalInput")
    d_part = nc.dram_tensor("part", [1, 8], f32, kind="ExternalOutput")
    d_pcols = nc.dram_tensor("pcols", [1, NCLS], f32, kind="ExternalOutput")

    with tile.TileContext(nc) as tc:
        with (
            tc.tile_pool(name="consts", bufs=1) as consts,
            tc.tile_pool(name="bigA", bufs=1) as bigA,
            tc.tile_pool(name="bigB", bufs=1) as bigB,
            tc.tile_pool(name="proj", bufs=1) as proj,
            tc.tile_pool(name="rows", bufs=1) as rows_pool,
            tc.tile_pool(name="wts", bufs=1) as wts,
            tc.tile_pool(name="work", bufs=1) as work,
            tc.tile_pool(name="simp", bufs=1) as simp,
            tc.tile_pool(name="maskp", bufs=2) as maskp,
            tc.tile_pool(name="junkp", bufs=2) as junkp,
            tc.tile_pool(name="small", bufs=8) as small,
            tc.tile_pool(name="acc", bufs=1) as accp,
        ):
            # ---- constants ----
            eye = consts.tile([128, 128], bf16, tag="eye")
            neye = consts.tile([128, 128], bf16, tag="neye")
            ones_bf = consts.tile([128, 1], bf16, tag="ones_bf")
            ones_f = consts.tile([128, 1], f32, tag="ones_f")
            negC = consts.tile([128, 1], f32, tag="negC")
            epsb = consts.tile([128, 1], f32, tag="epsb")
            nc.vector.memset(negC[:], -float(C_SENT))
            nc.vector.memset(epsb[:], float(EPS))
            f1b_sb = consts.tile([128, HT], f32, tag="f1b")
            f2b_sb = consts.tile([128, HT], f32, tag="f2b")
            nc.sync.dma_start(eye[:], d_eye[:, :])
            nc.sync.dma_start(neye[:], d_neye[:, :])
            nc.sync.dma_start(
                f1b_sb[:], d_f1b.ap().rearrange("(t p) o -> p (t o)", p=128)
            )
            nc.sync.dma_start(
                f2b_sb[:], d_f2b.ap().rearrange("(t p) o -> p (t o)", p=128)
            )
            nc.vector.memset(ones_bf[:], 1.0)
            nc.vector.memset(ones_f[:], 1.0)

            # ---- entropy part: column-partial softmax prob sums ----
            logits_sb = consts.tile([128, RBS, NCLS], f32, tag="logits")
            nc.sync.dma_start(
                logits_sb[:], d_log.ap().rearrange("(r p) c -> p r c", p=128)
            )

            # persistent accumulators
            acc8 = accp.tile([128, 8], f32, tag="acc8")
            nc.vector.memset(acc8[:], 0.0)

            with tc.tile_pool(name="psumE", bufs=1, space="PSUM") as psE:
                psP = psE.tile([1, NCLS], f32, tag="psP")
                for rb in range(RBS):
                    lg_rb = logits_sb[:, rb, :]
                    mxn = small.tile([128, 1], f32, tag="mxn")
                    nc.vector.reduce_max(mxn[:], lg_rb, axis=Ax.X, negate=True)
                    pe_t = junkp.tile([128, NCLS], f32, tag="pe")
                    ssum = small.tile([128, 1], f32, tag="esum")
                    nc.scalar.activation(
                        pe_t[:], lg_rb, Act.Exp, bias=mxn[:], scale=1.0,
                        accum_out=ssum[:],
                    )
                    rp = small.tile([128, 1], f32, tag="rp")
                    nc.vector.reciprocal(rp[:], ssum[:])
                    p_bf = junkp.tile([128, NCLS], bf16, tag="pbf")
                    nc.vector.tensor_scalar_mul(p_bf[:], pe_t[:], rp[:])
                    nc.tensor.matmul(
                        psP[:], ones_bf[:], p_bf[:],
                        start=(rb == 0), stop=(rb == RBS - 1),
                    )
                stageP = small.tile([1, NCLS], f32, tag="stageP")
                nc.scalar.copy(stageP[:], psP[:])
                nc.sync.dma_start(d_pcols[:, :], stageP[:])

            # ---- phase 0: projections f1_zT (own rows), f2_zT (all cols) ----
            zc = bigA.tile([128, KT, B], bf16, tag="bigA")     # raw z^T cols
            zr = rows_pool.tile([128, KT, R], bf16, tag="rowsA")
            f1wT_sb = wts.tile([128, KT, HID], bf16, tag="f1w")
            f2wT_sb = wts.tile([128, KT, HID], bf16, tag="f2w")
            nc.sync.dma_start(zc[:], d_zc.ap().rearrange("(t p) c -> p t c", p=128))
            nc.sync.dma_start(zr[:], d_zr.ap().rearrange("(t p) c -> p t c", p=128))
            nc.sync.dma_start(
                f1wT_sb[:], d_f1wT.ap().rearrange("(t p) c -> p t c", p=128)
            )
            nc.sync.dma_start(
                f2wT_sb[:], d_f2wT.ap().rearrange("(t p) c -> p t c", p=128)
            )

            f1zT = proj.tile([128, HT, R], bf16, tag="f1zT")
            f2zT = proj.tile([128, HT, B], bf16, tag="f2zT")

            with tc.tile_pool(name="psum0", bufs=2, space="PSUM") as ps0:
                for h in range(HT):
                    ps = ps0.tile([128, R], f32, tag="ps0")
                    for k in range(KT):
                        nc.tensor.matmul(
                            ps[:], f1wT_sb[:, k, h * 128:(h + 1) * 128],
                            zr[:, k, :], start=(k == 0), stop=(k == KT - 1),
                        )
                    nc.scalar.activation(
                        f1zT[:, h, :], ps[:], Act.Identity,
                        bias=f1b_sb[:, h:h + 1], scale=1.0,
                    )
                for h in range(HT):
                    for ct in range(CT):
                        cs = slice(ct * 512, (ct + 1) * 512)
                        ps = ps0.tile([128, 512], f32, tag="ps0")
                        for k in range(KT):
                            nc.tensor.matmul(
                                ps[:], f2wT_sb[:, k, h * 128:(h + 1) * 128],
                                zc[:, k, cs], start=(k == 0), stop=(k == KT - 1),
                            )
                        nc.scalar.activation(
                            f2zT[:, h, cs], ps[:], Act.Identity,
                            bias=f2b_sb[:, h:h + 1], scale=1.0,
                        )

            # ---- load normalized feature matrices (zc slot is reused) ----
            znc = bigA.tile([128, KT, B], bf16, tag="bigA")
            bnc = bigB.tile([128, KT, B], bf16, tag="bigB")
            znr = rows_pool.tile([128, KT, R], bf16, tag="rowsA")
            bnr = rows_pool.tile([128, KT, R], bf16, tag="rowsB")
            nc.sync.dma_start(znc[:], d_znc.ap().rearrange("(t p) c -> p t c", p=128))
            nc.sync.dma_start(bnc[:], d_bnc.ap().rearrange("(t p) c -> p t c", p=128))
            nc.sync.dma_start(znr[:], d_znr.ap().rearrange("(t p) c -> p t c", p=128))
            nc.sync.dma_start(bnr[:], d_bnr.ap().rearrange("(t p) c -> p t c", p=128))

            # ---- main loop over row blocks ----
            with (
                tc.tile_pool(name="psB", bufs=2, space="PSUM") as psBp,
                tc.tile_pool(name="psS", bufs=2, space="PSUM") as psSp,
                tc.tile_pool(name="psA", bufs=2, space="PSUM") as psAp,
            ):
                for rb in range(RBS):
                    rs = slice(rb * 128, (rb + 1) * 128)
                    sim_bf = simp.tile([128, B], bf16, tag="sim")
                    mask_bf = maskp.tile([128, B], bf16, tag="mask")
                    am = work.tile([128, B], f32, tag="am")
                    e_bf = work.tile([128, B], bf16, tag="ebf")
                    nn8 = small.tile([128, CT], f32, tag="nn8")

                    for ct in range(CT):
                        cs = slice(ct * 512, (ct + 1) * 512)
                        psB = psBp.tile([128, 512], f32, tag="psB")
                        psS = psSp.tile([128, 512], f32, tag="psS")
                        psA = psAp.tile([128, 512], f32, tag="psA")
                        for k in range(KT):
                            nc.tensor.matmul(
                                psB[:], bnr[:, k, rs], bnc[:, k, cs],
                                start=(k == 0), stop=(k == KT - 1),
                            )
                        for k in range(KT):
                            nc.tensor.matmul(
                                psS[:], znr[:, k, rs], znc[:, k, cs],
                                start=(k == 0), stop=(k == KT - 1),
                            )
                        for h in range(HT):
                            nc.tensor.matmul(
                                psA[:], f1zT[:, h, rs], f2zT[:, h, cs],
                                start=(h == 0), stop=(h == HT - 1),
                            )
                        # mask = (base_sim > eps), nn8[ct] = rowsum(mask)
                        nc.vector.tensor_scalar(
                            mask_bf[:, cs], psB[:], float(EPSILON), None,
                            op0=Alu.is_gt, op1=Alu.add,
                            accum_out=nn8[:, ct:ct + 1],
                        )
                        # sim tile: psum -> sbuf bf16 (ScalarE)
                        nc.scalar.copy(sim_bf[:, cs], psS[:])
                        # am = (attn + C) * mask   (fused psum drain + mask)
                        nc.vector.scalar_tensor_tensor(
                            am[:, cs], psA[:], float(C_SENT), mask_bf[:, cs],
                            op0=Alu.add, op1=Alu.mult,
                        )

                    # zero the diagonal 128x128 block of am (local col rb*128)
                    ds = slice(rb * 128, rb * 128 + 128)
                    nc.vector.tensor_tensor(
                        am[:, ds], am[:, ds], neye[:], op=Alu.mult
                    )

                    # e' = exp(am - C); S1 = rowsum(e')  [masked-only survives]
                    S1 = small.tile([128, 1], f32, tag="S1")
                    nc.scalar.activation(
                        e_bf[:], am[:], Act.Exp, bias=negC[:], scale=1.0,
                        accum_out=S1[:],
                    )
                    # T1 = rowsum(e' * sim)   (streamed over the am slot)
                    T1 = small.tile([128, 1], f32, tag="T1")
                    nc.vector.scalar_tensor_tensor(
                        am[:], e_bf[:], 1.0, sim_bf[:],
                        op0=Alu.mult, op1=Alu.mult, accum_out=T1[:],
                    )
                    # diag of sim (bf16-exact) for the denominator correction
                    dsim = small.tile([128, 1], f32, tag="dsim")
                    junk = junkp.tile([128, 128], bf16, tag="junk")
                    nc.vector.scalar_tensor_tensor(
                        junk[:], sim_bf[:, ds], 1.0, eye[:],
                        op0=Alu.mult, op1=Alu.mult, accum_out=dsim[:],
                    )
                    # es = exp(sim/tau), ssum = rowsum (streamed over am slot)
                    ssum = small.tile([128, 1], f32, tag="ssum")
                    nc.scalar.activation(
                        am[:], sim_bf[:], Act.Exp, bias=0.0, scale=1.0 / TAU,
                        accum_out=ssum[:],
                    )
                    dexp = small.tile([128, 1], f32, tag="dexp")
                    nc.scalar.activation(
                        dexp[:], dsim[:], Act.Exp, bias=0.0, scale=1.0 / TAU
                    )
                    denom = small.tile([128, 1], f32, tag="denom")
                    nc.vector.tensor_tensor(
                        denom[:], ssum[:], dexp[:], op=Alu.subtract
                    )
                    lg = small.tile([128, 1], f32, tag="lg")
                    nc.scalar.activation(
                        lg[:], denom[:], Act.Ln, bias=epsb[:], scale=1.0
                    )
                    # nn = rowsum(mask) - 1 (diagonal always passes threshold)
                    nn = small.tile([128, 1], f32, tag="nn")
                    nc.vector.tensor_reduce(
                        nn[:], nn8[:], axis=Ax.X, op=Alu.add
                    )
                    nc.vector.tensor_scalar_add(nn[:], nn[:], -1.0)
                    # loss_i = lg - (1/tau) * T1 / max(S1, tiny)
                    S1c = small.tile([128, 1], f32, tag="S1c")
                    nc.vector.tensor_scalar_max(S1c[:], S1[:], 1e-20)
                    rS1 = small.tile([128, 1], f32, tag="rS1")
                    nc.vector.reciprocal(rS1[:], S1c[:])
                    t0 = small.tile([128, 1], f32, tag="t0")
                    nc.vector.scalar_tensor_tensor(
                        t0[:], T1[:], -1.0 / TAU, rS1[:],
                        op0=Alu.mult, op1=Alu.mult,
                    )
                    loss_i = small.tile([128, 1], f32, tag="loss_i")
                    nc.vector.tensor_tensor(loss_i[:], lg[:], t0[:], op=Alu.add)
                    # contrib = loss_i / max(nn,1) * (nn > 0.5); valid flag
                    nnc = small.tile([128, 1], f32, tag="nnc")
                    nc.vector.tensor_scalar_max(nnc[:], nn[:], 1.0)
                    rnn = small.tile([128, 1], f32, tag="rnn")
                    nc.vector.reciprocal(rnn[:], nnc[:])
                    nc.vector.tensor_scalar(
                        acc8[:, 4 + rb:5 + rb], nn[:], 0.5, None, op0=Alu.is_gt
                    )
                    nc.vector.scalar_tensor_tensor(
                        acc8[:, rb:rb + 1], loss_i[:], rnn[:],
                        acc8[:, 4 + rb:5 + rb], op0=Alu.mult, op1=Alu.mult,
                    )

            # ---- final partition reduction via ones-matmul ----
            with tc.tile_pool(name="psumF", bufs=1, space="PSUM") as psF_pool:
                psF = psF_pool.tile([1, 8], f32, tag="psF")
                nc.tensor.matmul(psF[:], ones_f[:], acc8[:], start=True, stop=True)
                stageF = small.tile([1, 8], f32, tag="stageF")
                nc.scalar.copy(stageF[:], psF[:])
                nc.sync.dma_start(d_part[:, :], stageF[:])

    nc.compile()
    return nc


def _get_module():
    if "nc" not in _CACHE:
        _CACHE["nc"] = _build_module()
    return _CACHE["nc"]


def _l2norm_rows(x):
    n = np.linalg.norm(x.astype(np.float32), axis=1, keepdims=True)
    return x / np.maximum(n, 1e-12)


def build_in_maps(z_u, logits, old_class_indices, new_class_indices,
                  base_features, f1_w, f1_b, f2_w, f2_b):
    z_u = np.asarray(z_u, dtype=np.float32)
    base_features = np.asarray(base_features, dtype=np.float32)
    logits = np.asarray(logits, dtype=np.float32)

    znT = np.ascontiguousarray(_l2norm_rows(z_u).T.astype(BF))        # [D, B]
    bnT = np.ascontiguousarray(_l2norm_rows(base_features).T.astype(BF))
    zT = np.ascontiguousarray(z_u.T.astype(BF))
    f1wT = np.ascontiguousarray(np.asarray(f1_w, np.float32).T.astype(BF))
    f2wT = np.ascontiguousarray(np.asarray(f2_w, np.float32).T.astype(BF))
    f1b_c = np.asarray(f1_b, np.float32).reshape(HID, 1)
    f2b_c = np.asarray(f2_b, np.float32).reshape(HID, 1)
    active = np.concatenate(
        [np.asarray(old_class_indices), np.asarray(new_class_indices)]
    ).astype(np.int64)
    logits_act = np.ascontiguousarray(logits[:, active].astype(np.float32))
    eye = np.eye(128, dtype=BF)
    neye = (np.ones((128, 128), np.float32)
            - np.eye(128, dtype=np.float32)).astype(BF)

    in_maps = []
    for c in range(N_CORES):
        rs = slice(c * R, (c + 1) * R)
        in_maps.append({
            "zn_cols": np.ascontiguousarray(np.roll(znT, -c * R, axis=1)),
            "bn_cols": np.ascontiguousarray(np.roll(bnT, -c * R, axis=1)),
            "z_cols": np.ascontiguousarray(np.roll(zT, -c * R, axis=1)),
            "zn_rows": np.ascontiguousarray(znT[:, rs]),
            "bn_rows": np.ascontiguousarray(bnT[:, rs]),
            "z_rows": np.ascontiguousarray(zT[:, rs]),
            "f1wT": f1wT, "f2wT": f2wT, "f1b": f1b_c, "f2b": f2b_c,
            "logits_r": np.ascontiguousarray(logits_act[rs]),
            "eye128": eye, "oneminuseye": neye,
        })
    return in_maps


def combine_outputs(results, num_old, num_new):
    ps_sum = np.float32(0.0)
    nval = np.float32(0.0)
    colsum = np.zeros(NCLS, np.float32)
    for r in results:
        part = np.asarray(r["part"], np.float32).reshape(-1)
        ps_sum += part[0:4].sum(dtype=np.float32)
        nval += part[4:8].sum(dtype=np.float32)
        colsum += np.asarray(r["pcols"], np.float32).reshape(-1)

    loss_contr = np.float32(ps_sum / max(nval, np.float32(1.0))) \
        if nval > 0 else np.float32(0.0)

    mean_probs = (colsum / np.float32(B)).astype(np.float32)
    p_old = mean_probs[:num_old].sum(dtype=np.float32)
    p_new = mean_probs[num_old:].sum(dtype=np.float32)
    loss_inter = (p_old * np.log(p_old + np.float32(EPS))
                  + p_new * np.log(p_new + np.float32(EPS))
                  + np.float32(math.log(2.0)))
    p_old_in = mean_probs[:num_old] / (p_old + np.float32(EPS))
    loss_old_in = (p_old_in * np.log(p_old_in + np.float32(EPS))).sum(
        dtype=np.float32) + np.float32(math.log(num_old))
    p_new_in = mean_probs[num_old:] / (p_new + np.float32(EPS))
    if num_new > 1:
        loss_new_in = (p_new_in * np.log(p_new_in + np.float32(EPS))).sum(
            dtype=np.float32) + np.float32(math.log(num_new))
    else:
        loss_new_in = np.float32(0.0)
    total = np.float32(loss_inter + loss_old_in + loss_new_in + loss_contr)
    return np.array(total, dtype=np.float32)


def kernel(z_u, logits, old_class_indices, new_class_indices, base_features,
           f1_w, f1_b, f2_w, f2_b):
    from concourse.bass_utils import run_bass_kernel_spmd

    nc = _get_module()
    in_maps = build_in_maps(z_u, logits, old_class_indices, new_class_indices,
                            base_features, f1_w, f1_b, f2_w, f2_b)
    res = run_bass_kernel_spmd(nc, in_maps, core_ids=list(range(N_CORES)))
    return combine_outputs(
        res.results,
        num_old=len(np.asarray(old_class_indices)),
        num_new=len(np.asarray(new_class_indices)),
    )
